# revision 30
# baseline (speedup 1.0000x reference)
"""LocallyConnected2d Bass kernel for 8 TRN2 NeuronCores.

Problem: out[b,o,oh,ow] = sum_{c,kh,kw} x[b,c,oh+kh-1,ow+kw-1] * w[o,c,oh,ow,kh*3+kw]
Shapes: x (8,64,32,32) f32, weight (1,64,64,32,32,9) f32 -> out (8,64,32,32) f32.

Sharding: each core owns 4 consecutive output rows (oh); the 151 MiB weight
tensor is read exactly once, 1 byte/elem (int8), with no duplication and no
collectives.

Numerics: weights are quantized to int8 with one global scale s_g =
max|w|/127; s_g is folded into x on the host (x*s_g in bf16), so the device
only upcasts int8->bf16 (exact) and the matmul runs in bf16 with fp32 PSUM
accumulation. Measured end-to-end max rel err 1.3e-2 (tolerance 2e-2).

Per-core kernel: every output location is an independent tiny matmul
  out_loc[b, o] = patches_loc[ck, b].T @ w_loc[ck, o]
PSUM-accumulated over tap groups (M=b=8, N=o=64). The 9 taps pack into 5
matmuls per location ("tap pairing"): partitions hold (tapA c | tapB c),
where partitions 64-127 of the x tile carry a pre-shifted copy of the input.
Pairs (0,1)(3,4)(6,7) use a (0,+1)-column-shifted copy, pair (2,5) a
(+1,0)-row-shifted copy, and tap 8 of an even/odd column pair shares one
128-partition tile (K=64 matmuls on each half).

Perf structure vs v1:
- All matmul operands are contiguous in SBUF: weights laid out with o
  innermost ([oh_l, p, slot, owp, o]), x with b innermost ([p, h, w, b]).
- 4-way tensor-engine column tiling: location ow -> column group j = ow%4;
  group j's 5-matmul chain accumulates into PSUM bank j at partitions
  32j..32j+8, so 4 chains stream concurrently through disjoint 32-column
  strips of the PE array.
- Weights ship as int8 (4.7 MB/core) and are upcast to bf16 by one DVE
  tensor_copy per row tile; PSUM is drained by the Scalar engine into an
  f16 out tile (cast during copy), DMA'd out per row.
"""

import numpy as np
import ml_dtypes

import concourse.bacc as bacc
import concourse.bass as bass
import concourse.tile as tile
from concourse import mybir
from concourse.bass_utils import run_bass_kernel_spmd

B, C, O = 8, 64, 64
OH, OW = 32, 32
NCORES = 8
R = OH // NCORES          # 4 oh rows per core
HS = R + 2                # x halo rows per core
WS = OW + 2               # padded width
F32 = mybir.dt.float32
F16 = mybir.dt.float16
BF16 = mybir.dt.bfloat16
I8 = mybir.dt.int8

# Tap pairing: slots 0-3 are (tapA, tapB) pairs; taps are k = 3*kh + kw.
PAIRS = [(0, 1), (3, 4), (6, 7), (2, 5)]
# lhsT base (kh, kw, which x tile) per pair slot; x tile 0 = column-shifted
# duplicate in partitions 64+, tile 1 = row-shifted duplicate.
PAIR_BASE = [(0, 0, 0), (1, 0, 0), (2, 0, 0), (0, 2, 1)]

_cache: dict = {}
_last_in_maps = None


def _build() -> bass.Bass:
    nc = bacc.Bacc("TRN2", target_bir_lowering=False, debug=False,
                   num_devices=NCORES)
    # x patches, b innermost: [0:64] = slab [c,h,w,b]; [64:128] = shifted dup.
    xa = nc.dram_tensor("xa", [128, HS, WS, B], BF16, kind="ExternalInput").ap()
    xb = nc.dram_tensor("xb", [128, HS, WS, B], BF16, kind="ExternalInput").ap()
    # Weights: [oh_l, chunk, p, slot, owp_local, o], chunk-contiguous.
    # Chunks 0-2 ship int8 (device upcast); chunk 3 — consumed last within
    # each row — ships bf16, trimming upcast work by 25%.
    NCH = 4                    # owp chunks per row
    CW = (OW // 2) // NCH      # owp per chunk
    ws = nc.dram_tensor("ws", [R, NCH - 1, 128, 9, CW, O], I8,
                        kind="ExternalInput").ap()
    wsb = nc.dram_tensor("wsb", [R, 128, 9, CW, O], BF16,
                         kind="ExternalInput").ap()
    out = nc.dram_tensor("out", [R, 128, 8, O], F16, kind="ExternalOutput").ap()

    with tile.TileContext(nc) as tc:
        with (
            tc.tile_pool(name="xpool", bufs=1) as xpool,
            tc.tile_pool(name="wqpool", bufs=3 * (NCH - 1)) as wqpool,
            tc.tile_pool(name="wbpool", bufs=3 * NCH) as wbpool,
            tc.tile_pool(name="opool", bufs=2) as opool,
            tc.tile_pool(name="pspool", bufs=8, space="PSUM") as pspool,
        ):
            x_sb = [xpool.tile([128, HS, WS, B], BF16, name="xa_sb"),
                    xpool.tile([128, HS, WS, B], BF16, name="xb_sb")]

            # Weight chunk load (int8) + upcast, spread over three engines:
            # DVE carries most of it, GpSimd half of chunk 1, Scalar a
            # quarter of chunk 3 (it also owns the PSUM drains).
            wq = {}
            wb = {}
            def w_load(r, c):
                wb[r, c] = wbpool.tile([128, 9, CW, O], BF16, tag="wb",
                                       name=f"wb_{r}_{c}")
                if c == 3:
                    nc.sync.dma_start(wb[r, c][:], wsb[r])
                    return
                wq[r, c] = wqpool.tile([128, 9, CW, O], I8, tag="wq",
                                       name=f"wq_{r}_{c}")
                nc.sync.dma_start(wq[r, c][:], ws[r, c])
                if c == 2:
                    nc.vector.tensor_copy(out=wb[r, c][:, :, 0:2, :],
                                          in_=wq[r, c][:, :, 0:2, :])
                    nc.scalar.copy(out=wb[r, c][:, :, 2:4, :],
                                   in_=wq[r, c][:, :, 2:4, :])
                else:
                    nc.vector.tensor_copy(out=wb[r, c][:], in_=wq[r, c][:])

            # PE warm-up: dummy matmuls on scratch data release the HAM
            # clock gate (~3.4us of activity) before the real stream.
            scr = xpool.tile([128, O], BF16, name="scr")
            nc.vector.memset(scr[:], 0)
            warm = pspool.tile([128, 8, O], F32, tag="ps", name="warm")
            for _ in range(48):
                nc.tensor.matmul(warm[0:B, 0, :], scr[:, 0:B], scr[:, :],
                                 start=True, stop=True, tile_position=(0, 0))

            w_load(0, 0)
            nc.sync.dma_start(x_sb[0][:], xa)
            nc.sync.dma_start(x_sb[1][:], xb)
            for c in range(1, NCH):
                w_load(0, c)
            for c in range(NCH):
                w_load(1, c)

            for oh_l in range(R):
                ps = [pspool.tile([128, 8, O], F32, tag="ps",
                                  name=f"ps_{oh_l}_{j}") for j in range(4)]
                ot = opool.tile([128, 8, O], F16, tag="ot")

                for s in range(8):
                    wbc = wb[oh_l, s // 2]
                    for t in range(5):
                        for j in range(4):
                            ow = 4 * s + j
                            eo = ow % 2
                            owp_l = 2 * (s % 2) + j // 2
                            po = ps[j][32 * j:32 * j + B, s, :]
                            if t < 4:
                                kh, kw, xt = PAIR_BASE[t]
                                lhsT = x_sb[xt][:, oh_l + kh, ow + kw, :]
                                rhs = wbc[:, 4 * eo + t, owp_l, :]
                            elif eo == 0:  # tap 8 via unshifted half
                                lhsT = x_sb[0][0:64, oh_l + 2, ow + 2, :]
                                rhs = wbc[0:64, 8, owp_l, :]
                            else:          # tap 8 via column-shifted half
                                lhsT = x_sb[0][64:128, oh_l + 2, ow + 1, :]
                                rhs = wbc[64:128, 8, owp_l, :]
                            row_base = 64 if (t == 4 and eo == 1) else 0
                            nc.tensor.matmul(po, lhsT, rhs,
                                             start=(t == 0), stop=(t == 4),
                                             tile_position=(row_base, 32 * j))

                last = oh_l == R - 1
                for j in range(4):
                    dst = ot[32 * j:32 * j + B, :, :]
                    src = ps[j][32 * j:32 * j + B, :]
                    if last and j < 2:   # split the tail drain over 2 engines
                        nc.vector.tensor_copy(out=dst, in_=src)
                    else:
                        nc.scalar.copy(out=dst, in_=src)
                nc.sync.dma_start(out[oh_l], ot[:])

                if oh_l + 2 < R:   # prefetch two rows ahead (after drains,
                    for c in range(NCH):   # so drains win engine-queue order)
                        w_load(oh_l + 2, c)
    nc.compile()
    return nc


def _marshal(x: np.ndarray, weight: np.ndarray) -> list[dict]:
    x = np.ascontiguousarray(x, dtype=np.float32)
    w = weight[0]  # (O, C, OH, OW, K)

    sg = float(np.abs(w).max()) / 127.0
    q = np.clip(np.round(w / sg), -127, 127).astype(np.int8)

    # Fold the weight scale into x; pad H and W.
    xs = (x * sg).astype(ml_dtypes.bfloat16)
    xp = np.zeros((B, C, OH + 2, OW + 2), dtype=ml_dtypes.bfloat16)
    xp[:, :, 1:OH + 1, 1:OW + 1] = xs

    in_maps = []
    for r in range(NCORES):
        # slab [c, h, w, b], b innermost
        slab = xp[:, :, R * r:R * r + HS, :].transpose(1, 2, 3, 0)
        sw = np.zeros_like(slab)
        sw[:, :, :WS - 1, :] = slab[:, :, 1:, :]        # column shift
        sh = np.zeros_like(slab)
        sh[:, :HS - 1, :, :] = slab[:, 1:, :, :]        # row shift
        xa_r = np.concatenate([slab, sw], axis=0)
        xb_r = np.concatenate([slab, sh], axis=0)

        # weight slab -> [oh_l, p, slot, owp, o]
        wt = q[:, :, R * r:R * (r + 1), :, :].transpose(2, 1, 0, 3, 4)
        # wt: [oh, c, o, ow, k]
        even, odd = wt[:, :, :, 0::2, :], wt[:, :, :, 1::2, :]
        W2 = np.empty((R, 128, 9, OW // 2, O), dtype=np.int8)
        for s, (ka, kb) in enumerate(PAIRS):
            W2[:, 0:64, s] = even[..., ka].transpose(0, 1, 3, 2)
            W2[:, 64:128, s] = even[..., kb].transpose(0, 1, 3, 2)
            W2[:, 0:64, 4 + s] = odd[..., ka].transpose(0, 1, 3, 2)
            W2[:, 64:128, 4 + s] = odd[..., kb].transpose(0, 1, 3, 2)
        W2[:, 0:64, 8] = even[..., 8].transpose(0, 1, 3, 2)
        W2[:, 64:128, 8] = odd[..., 8].transpose(0, 1, 3, 2)
        # -> [oh_l, chunk, p, slot, owp_local, o]
        W3 = W2.reshape(R, 128, 9, 4, 4, O).transpose(0, 3, 1, 2, 4, 5)
        in_maps.append({
            "xa": np.ascontiguousarray(xa_r),
            "xb": np.ascontiguousarray(xb_r),
            "ws": np.ascontiguousarray(W3[:, :3]),
            "wsb": np.ascontiguousarray(
                W3[:, 3].astype(ml_dtypes.bfloat16)),
        })
    return in_maps


def kernel(x: np.ndarray, weight: np.ndarray) -> np.ndarray:
    global _last_in_maps
    in_maps = _marshal(x, weight)
    _last_in_maps = in_maps

    if "nc" not in _cache:
        _cache["nc"] = _build()
    res = run_bass_kernel_spmd(_cache["nc"], in_maps, list(range(NCORES)))

    # Per-core out is [R, 128, 8, O] f16 with partition 32j+b, free (s, o);
    # location ow = 4s + j. Stitch to (B, O, OH, OW).
    full = np.empty((B, O, OH, OW), dtype=np.float32)
    for r in range(NCORES):
        o_np = np.asarray(res.results[r]["out"], dtype=np.float32)
        o_np = o_np.reshape(R, 4, 32, 8, O)[:, :, :B]  # [r, j, b, s, o]
        # -> (b, o, oh_l, s, j)
        full[:, :, R * r:R * (r + 1), :] = (
            o_np.transpose(2, 4, 0, 3, 1).reshape(B, O, R, OW))
    return np.ascontiguousarray(full)


# revision 34
# speedup vs baseline: 1.0744x; 1.0744x over previous
"""LocallyConnected2d Bass kernel for 8 TRN2 NeuronCores.

Problem: out[b,o,oh,ow] = sum_{c,kh,kw} x[b,c,oh+kh-1,ow+kw-1] * w[o,c,oh,ow,kh*3+kw]
Shapes: x (8,64,32,32) f32, weight (1,64,64,32,32,9) f32 -> out (8,64,32,32) f32.

Sharding: each core owns 4 consecutive output rows (oh); the 151 MiB weight
tensor is read exactly once, 1 byte/elem (int8), with no duplication and no
collectives.

Numerics: weights are quantized to int8 with one global scale s_g =
max|w|/127; s_g is folded into x on the host (x*s_g in bf16), so the device
only upcasts int8->bf16 (exact) and the matmul runs in bf16 with fp32 PSUM
accumulation. Measured end-to-end max rel err 1.3e-2 (tolerance 2e-2).

Per-core kernel: every output location is an independent tiny matmul
  out_loc[b, o] = patches_loc[ck, b].T @ w_loc[ck, o]
PSUM-accumulated over tap groups (M=b=8, N=o=64). The 9 taps pack into 5
matmuls per location ("tap pairing"): partitions hold (tapA c | tapB c),
where partitions 64-127 of the x tile carry a pre-shifted copy of the input.
Pairs (0,1)(3,4)(6,7) use a (0,+1)-column-shifted copy, pair (2,5) a
(+1,0)-row-shifted copy, and tap 8 of an even/odd column pair shares one
128-partition tile (K=64 matmuls on each half).

Perf structure vs v1:
- All matmul operands are contiguous in SBUF: weights laid out with o
  innermost ([oh_l, p, slot, owp, o]), x with b innermost ([p, h, w, b]).
- 4-way tensor-engine column tiling: location ow -> column group j = ow%4;
  group j's 5-matmul chain accumulates into PSUM bank j at partitions
  32j..32j+8, so 4 chains stream concurrently through disjoint 32-column
  strips of the PE array.
- Weights ship as int8 (4.7 MB/core) and are upcast to bf16 by one DVE
  tensor_copy per row tile; PSUM is drained by the Scalar engine into an
  f16 out tile (cast during copy), DMA'd out per row.
"""

import numpy as np
import ml_dtypes

import concourse.bacc as bacc
import concourse.bass as bass
import concourse.tile as tile
from concourse import mybir
from concourse.bass_utils import run_bass_kernel_spmd

B, C, O = 8, 64, 64
OH, OW = 32, 32
NCORES = 8
R = OH // NCORES          # 4 oh rows per core
HS = R + 2                # x halo rows per core
WS = OW + 2               # padded width
F32 = mybir.dt.float32
F16 = mybir.dt.float16
BF16 = mybir.dt.bfloat16
I8 = mybir.dt.int8

# Tap pairing: slots 0-3 are (tapA, tapB) pairs; taps are k = 3*kh + kw.
PAIRS = [(0, 1), (3, 4), (6, 7), (2, 5)]
# lhsT base (kh, kw, which x tile) per pair slot; x tile 0 = column-shifted
# duplicate in partitions 64+, tile 1 = row-shifted duplicate.
PAIR_BASE = [(0, 0, 0), (1, 0, 0), (2, 0, 0), (0, 2, 1)]

_cache: dict = {}
_last_in_maps = None


def _build() -> bass.Bass:
    nc = bacc.Bacc("TRN2", target_bir_lowering=False, debug=False,
                   num_devices=NCORES)
    # x patches, b innermost: [0:64] = slab [c,h,w,b]; [64:128] = shifted dup.
    xa = nc.dram_tensor("xa", [128, HS, WS, B], BF16, kind="ExternalInput").ap()
    xb = nc.dram_tensor("xb", [128, HS, WS, B], BF16, kind="ExternalInput").ap()
    # Weights: [oh_l, chunk, p, slot, owp_local, o], chunk-contiguous int8.
    NCH = 4                    # owp chunks per row
    CW = (OW // 2) // NCH      # owp per chunk
    ws = nc.dram_tensor("ws", [R, NCH, 128, 9, CW, O], I8,
                        kind="ExternalInput").ap()
    out = nc.dram_tensor("out", [R, 128, 8, O], F16, kind="ExternalOutput").ap()

    with tile.TileContext(nc) as tc:
        with (
            tc.tile_pool(name="xpool", bufs=1) as xpool,
            tc.tile_pool(name="wqpool", bufs=3 * NCH) as wqpool,
            tc.tile_pool(name="wbpool", bufs=3 * NCH) as wbpool,
            tc.tile_pool(name="opool", bufs=2) as opool,
            tc.tile_pool(name="pspool", bufs=8, space="PSUM") as pspool,
        ):
            x_sb = [xpool.tile([128, HS, WS, B], BF16, name="xa_sb"),
                    xpool.tile([128, HS, WS, B], BF16, name="xb_sb")]

            # Weight chunk load (int8) + upcast, spread over three engines:
            # DVE carries most of it, GpSimd half of chunk 1, Scalar a
            # quarter of chunk 3 (it also owns the PSUM drains).
            wq = {}
            wb = {}
            def w_load(r, c):
                wb[r, c] = wbpool.tile([128, 9, CW, O], BF16, tag="wb",
                                       name=f"wb_{r}_{c}")
                wq[r, c] = wqpool.tile([128, 9, CW, O], I8, tag="wq",
                                       name=f"wq_{r}_{c}")
                nc.sync.dma_start(wq[r, c][:], ws[r, c])
                if c == 3:
                    nc.vector.tensor_copy(out=wb[r, c][:, :, 0:2, :],
                                          in_=wq[r, c][:, :, 0:2, :])
                    nc.scalar.copy(out=wb[r, c][:, :, 2:4, :],
                                   in_=wq[r, c][:, :, 2:4, :])
                else:
                    nc.vector.tensor_copy(out=wb[r, c][:], in_=wq[r, c][:])

            # PE warm-up: dummy matmuls on scratch data release the HAM
            # clock gate (~3.4us of activity) before the real stream.
            scr = xpool.tile([128, O], BF16, name="scr")
            nc.vector.memset(scr[:], 0)
            warm = pspool.tile([128, 8, O], F32, tag="ps", name="warm")
            for _ in range(48):
                nc.tensor.matmul(warm[0:B, 0, :], scr[:, 0:B], scr[:, :],
                                 start=True, stop=True, tile_position=(0, 0))

            w_load(0, 0)
            nc.sync.dma_start(x_sb[0][:], xa)
            nc.sync.dma_start(x_sb[1][:], xb)
            for c in range(1, NCH):
                w_load(0, c)
            for c in range(NCH):
                w_load(1, c)

            for oh_l in range(R):
                ps = [pspool.tile([128, 8, O], F32, tag="ps",
                                  name=f"ps_{oh_l}_{j}") for j in range(4)]
                ot = opool.tile([128, 8, O], F16, tag="ot")

                for s in range(8):
                    wbc = wb[oh_l, s // 2]
                    for t in range(5):
                        for j in range(4):
                            ow = 4 * s + j
                            eo = ow % 2
                            owp_l = 2 * (s % 2) + j // 2
                            po = ps[j][32 * j:32 * j + B, s, :]
                            if t < 4:
                                kh, kw, xt = PAIR_BASE[t]
                                lhsT = x_sb[xt][:, oh_l + kh, ow + kw, :]
                                rhs = wbc[:, 4 * eo + t, owp_l, :]
                            elif eo == 0:  # tap 8 via unshifted half
                                lhsT = x_sb[0][0:64, oh_l + 2, ow + 2, :]
                                rhs = wbc[0:64, 8, owp_l, :]
                            else:          # tap 8 via column-shifted half
                                lhsT = x_sb[0][64:128, oh_l + 2, ow + 1, :]
                                rhs = wbc[64:128, 8, owp_l, :]
                            row_base = 64 if (t == 4 and eo == 1) else 0
                            nc.tensor.matmul(po, lhsT, rhs,
                                             start=(t == 0), stop=(t == 4),
                                             tile_position=(row_base, 32 * j))

                last = oh_l == R - 1
                for j in range(4):
                    dst = ot[32 * j:32 * j + B, :, :]
                    src = ps[j][32 * j:32 * j + B, :]
                    if last and j < 2:   # split the tail drain over 2 engines
                        nc.vector.tensor_copy(out=dst, in_=src)
                    else:
                        nc.scalar.copy(out=dst, in_=src)
                nc.sync.dma_start(out[oh_l], ot[:])

                if oh_l + 2 < R:   # prefetch two rows ahead (after drains,
                    for c in range(NCH):   # so drains win engine-queue order)
                        w_load(oh_l + 2, c)
    nc.compile()
    return nc


def _marshal(x: np.ndarray, weight: np.ndarray) -> list[dict]:
    x = np.ascontiguousarray(x, dtype=np.float32)
    w = weight[0]  # (O, C, OH, OW, K)

    sg = float(np.abs(w).max()) / 127.0
    q = np.clip(np.round(w / sg), -127, 127).astype(np.int8)

    # Fold the weight scale into x; pad H and W.
    xs = (x * sg).astype(ml_dtypes.bfloat16)
    xp = np.zeros((B, C, OH + 2, OW + 2), dtype=ml_dtypes.bfloat16)
    xp[:, :, 1:OH + 1, 1:OW + 1] = xs

    in_maps = []
    for r in range(NCORES):
        # slab [c, h, w, b], b innermost
        slab = xp[:, :, R * r:R * r + HS, :].transpose(1, 2, 3, 0)
        sw = np.zeros_like(slab)
        sw[:, :, :WS - 1, :] = slab[:, :, 1:, :]        # column shift
        sh = np.zeros_like(slab)
        sh[:, :HS - 1, :, :] = slab[:, 1:, :, :]        # row shift
        xa_r = np.concatenate([slab, sw], axis=0)
        xb_r = np.concatenate([slab, sh], axis=0)

        # weight slab -> [oh_l, p, slot, owp, o]
        wt = q[:, :, R * r:R * (r + 1), :, :].transpose(2, 1, 0, 3, 4)
        # wt: [oh, c, o, ow, k]
        even, odd = wt[:, :, :, 0::2, :], wt[:, :, :, 1::2, :]
        W2 = np.empty((R, 128, 9, OW // 2, O), dtype=np.int8)
        for s, (ka, kb) in enumerate(PAIRS):
            W2[:, 0:64, s] = even[..., ka].transpose(0, 1, 3, 2)
            W2[:, 64:128, s] = even[..., kb].transpose(0, 1, 3, 2)
            W2[:, 0:64, 4 + s] = odd[..., ka].transpose(0, 1, 3, 2)
            W2[:, 64:128, 4 + s] = odd[..., kb].transpose(0, 1, 3, 2)
        W2[:, 0:64, 8] = even[..., 8].transpose(0, 1, 3, 2)
        W2[:, 64:128, 8] = odd[..., 8].transpose(0, 1, 3, 2)
        # -> [oh_l, chunk, p, slot, owp_local, o]
        W3 = W2.reshape(R, 128, 9, 4, 4, O).transpose(0, 3, 1, 2, 4, 5)
        in_maps.append({
            "xa": np.ascontiguousarray(xa_r),
            "xb": np.ascontiguousarray(xb_r),
            "ws": np.ascontiguousarray(W3),
        })
    return in_maps


def kernel(x: np.ndarray, weight: np.ndarray) -> np.ndarray:
    global _last_in_maps
    in_maps = _marshal(x, weight)
    _last_in_maps = in_maps

    if "nc" not in _cache:
        _cache["nc"] = _build()
    res = run_bass_kernel_spmd(_cache["nc"], in_maps, list(range(NCORES)))

    # Per-core out is [R, 128, 8, O] f16 with partition 32j+b, free (s, o);
    # location ow = 4s + j. Stitch to (B, O, OH, OW).
    full = np.empty((B, O, OH, OW), dtype=np.float32)
    for r in range(NCORES):
        o_np = np.asarray(res.results[r]["out"], dtype=np.float32)
        o_np = o_np.reshape(R, 4, 32, 8, O)[:, :, :B]  # [r, j, b, s, o]
        # -> (b, o, oh_l, s, j)
        full[:, :, R * r:R * (r + 1), :] = (
            o_np.transpose(2, 4, 0, 3, 1).reshape(B, O, R, OW))
    return np.ascontiguousarray(full)
